# revision 2
# baseline (speedup 1.0000x reference)
"""Column-wise RMS normalization on 8 Trainium2 NeuronCores — transposed layout.

Computes y = x * rsqrt(sum(x*x, axis=0) + eps) for x [32768, 2048] f32.

Sharding: column-parallel — each core owns 256 columns (no collectives).
The host casts to fp16 and TRANSPOSES the shard to [2 panels, 128, N]:
panel h of core k holds 128 columns as SBUF partitions, with the full
row axis (N=32768) as the free axis. Every DMA chunk [128, FC] moves
FC*2 bytes CONTIGUOUS per partition (8KB runs), and the column
reduction becomes a free-axis reduction: no TensorE, no PSUM.

Reduction per chunk: ACT ACTIVATE(Square, accum_out) in one pass
(panel B entirely: its chunks arrive at half rate, which ACT tracks
by itself), or DVE tensor_mul->fp8 + reduce_sum (2-pass; used for
half of panel A while DVE is otherwise idle). Chain: sqrt(u+eps) on
ACT + reciprocal on DVE on [128,1] — the scale is per-partition, so
output muls are DVE tensor_tensor with a [P,1] broadcast at full rate.

DMA: ALL transfers ride the single SP HWDGE queue in strict FIFO
program order [in-A, in-B, out-A, out-B]. A single queue sustains
~420 GB/s, and the FIFO order structurally hides all reduction+chain
latency: out-A's descriptors are consumed only after in-B drains
(~20us after panel A's data lands), out-B after out-A. HBM stays busy
start-to-finish; the only exposed overhead is the fixed runtime
preamble/postamble (~19us).
"""

import numpy as np

import concourse.bacc as bacc
import concourse.bass as bass
import concourse.tile as tile
from concourse import mybir
from concourse.bass_utils import run_bass_kernel_spmd

N, D = 32768, 2048
EPS = 1e-6
NCORES = 8
P = 128           # partitions = columns per panel
NPANEL = 2        # panels per core (256 cols)
FC = 4096         # free-axis chunk (1MB per DMA / square / mul chunk)
NCH = N // FC     # 8 chunks per panel
H2 = 128          # inner axis of the 3D op views (DVE 2x-rate mode needs
G2 = FC // H2     # a contiguous inner axis; stride-0 only on the middle)

# Square-chunk owners: "A" = ACT one-pass Square+accum, "V" = DVE one-pass
# scalar_tensor_tensor((x*1)*x, accum=sum). Both run at ~1 elem/cycle
# (ACT 3.7us, DVE 4.4us per 4096-chunk); DVE also runs all output muls
# (2.3us each, 2x mode), so it only takes early-arriving square chunks —
# the panel-B boundary chain is [last-sq -> m6/m7 -> foldB -> Bmul0].
OWN_A = ["V", "A", "A", "V", "A", "A", "V", "A"]
OWN_B = ["V", "A", "A", "V", "A", "A", "A", "A"]
# Output-mul owners: "V" = DVE broadcast mul, "A" = ACT Copy-with-scale.
MUL_A = ["V"] * NCH
MUL_B = ["V"] * NCH

_NC = None


def _build() -> bass.Bass:
    nc = bacc.Bacc("TRN2", target_bir_lowering=False, enable_partition_id=False)
    x = nc.dram_tensor("x", [NPANEL, P, N], mybir.dt.float16, kind="ExternalInput")
    y = nc.dram_tensor("y", [NPANEL, P, N], mybir.dt.float16, kind="ExternalOutput")

    with tile.TileContext(nc) as tc:
        with (
            tc.tile_pool(name="cache", bufs=1) as cachep,
            tc.tile_pool(name="consts", bufs=1) as consts,
            tc.tile_pool(name="sq", bufs=2) as sqp,
            tc.tile_pool(name="outs", bufs=4) as outp,
            tc.tile_pool(name="scale", bufs=1) as scalep,
        ):
            xc = cachep.tile([P, NPANEL, N], mybir.dt.float16)
            eps_t = consts.tile([P, 1], mybir.dt.float32)
            nc.vector.memset(eps_t, EPS)
            # Pre-warm ACT function tables off the critical path.
            warm = consts.tile([1, 1], mybir.dt.float32)
            nc.scalar.activation(
                out=warm, in_=eps_t[0:1, 0:1],
                func=mybir.ActivationFunctionType.Square,
            )
            nc.scalar.activation(
                out=warm, in_=eps_t[0:1, 0:1],
                func=mybir.ActivationFunctionType.Sqrt,
            )

            parts = scalep.tile([P, NPANEL, NCH], mybir.dt.float32)
            s32 = [scalep.tile([P, 1], mybir.dt.float32, name=f"s32_{h}")
                   for h in range(NPANEL)]
            # Scale replicated along a 128-wide inner axis so output muls can
            # broadcast over the MIDDLE axis with a contiguous inner axis
            # (DVE 2x fp16 mode), like the baseline's [P,1,H] pattern.
            smax = [scalep.tile([P, 1, H2], mybir.dt.float16, name=f"smax{h}")
                    for h in range(NPANEL)]

            def in_panel(h):
                for c in range(NCH):
                    cs = slice(c * FC, (c + 1) * FC)
                    nc.sync.dma_start(out=xc[:, h, cs], in_=x[h, :, cs])

            def sq_chunk(h, c, owner):
                cs = slice(c * FC, (c + 1) * FC)
                part = parts[:, h, c : c + 1]
                if owner == "A":
                    scr = sqp.tile([P, FC], mybir.dt.float16, tag="scrA", bufs=1)
                    nc.scalar.activation(
                        out=scr, in_=xc[:, h, cs],
                        func=mybir.ActivationFunctionType.Square,
                        accum_out=part,
                    )
                else:
                    xv = xc[:, h, cs].rearrange("p (g i) -> p g i", i=H2)
                    scr = sqp.tile([P, G2, H2], mybir.dt.float16, tag="scrV", bufs=1)
                    nc.vector.scalar_tensor_tensor(
                        out=scr, in0=xv, scalar=1.0, in1=xv,
                        op0=mybir.AluOpType.mult, op1=mybir.AluOpType.mult,
                        accum_out=part,
                    )

            def chain(h):
                u = scalep.tile([P, 1], mybir.dt.float32, name=f"u{h}")
                nc.vector.reduce_sum(
                    u, parts[:, h : h + 1, :], axis=mybir.AxisListType.X
                )
                t = scalep.tile([P, 1], mybir.dt.float32, name=f"t{h}")
                nc.scalar.activation(
                    out=t, in_=u,
                    func=mybir.ActivationFunctionType.Sqrt,
                    bias=eps_t[:, 0:1], scale=1.0,
                )
                nc.vector.reciprocal_approx_fast(out=s32[h], in_=t)
                nc.vector.tensor_copy(
                    smax[h], s32[h][:, 0:1].to_broadcast((P, 1, H2))
                )

            def out_chunk(h, c, owner, split=1):
                cs = slice(c * FC, (c + 1) * FC)
                ot = outp.tile([P, G2, H2], mybir.dt.float16, tag="ot", bufs=6)
                if owner == "A":
                    nc.scalar.activation(
                        out=ot, in_=xc[:, h, cs].rearrange("p (g i) -> p g i", i=H2),
                        func=mybir.ActivationFunctionType.Copy,
                        scale=s32[h][:, 0:1],
                    )
                else:
                    xv = xc[:, h, cs].rearrange("p (g i) -> p g i", i=H2)
                    nc.vector.tensor_mul(
                        ot, xv, smax[h][:, :, :].to_broadcast((P, G2, H2))
                    )
                otf = ot[:, :, :].rearrange("p g i -> p (g i)")
                # split>1 pipelines the final chunk's completion drain.
                fs = FC // split
                for k in range(split):
                    ks = slice(c * FC + k * fs, c * FC + (k + 1) * fs)
                    nc.sync.dma_start(out=y[h, :, ks], in_=otf[:, k * fs : (k + 1) * fs])

            # Panel A input + squares.
            in_panel(0)
            for c in range(NCH):
                sq_chunk(0, c, OWN_A[c])
            chain(0)
            # Panel B input (SP FIFO: behind all of in-A).
            in_panel(1)
            # Interleave panel-B squares with panel-A outputs.
            for c in range(NCH):
                sq_chunk(1, c, OWN_B[c])
                out_chunk(0, c, MUL_A[c])
            chain(1)
            # Panel B outputs (SP FIFO: behind out-A). The final chunk's DMA
            # is split so the end-of-kernel completion drain overlaps earlier
            # sub-transfers.
            for c in range(NCH):
                out_chunk(1, c, MUL_B[c], split=(4 if c == NCH - 1 else 1))
    nc.compile()
    return nc


def _get_nc() -> bass.Bass:
    global _NC
    if _NC is None:
        _NC = _build()
    return _NC


def _shard_inputs(x: np.ndarray) -> list[dict]:
    xt = np.ascontiguousarray(x.astype(np.float16).T)  # [D, N]
    out = []
    for i in range(NCORES):
        sh = xt[i * NPANEL * P : (i + 1) * NPANEL * P]  # [256, N]
        out.append({"x": sh.reshape(NPANEL, P, N)})
    return out


def kernel(x) -> np.ndarray:
    x = np.asarray(x, dtype=np.float32)
    assert x.shape == (N, D), x.shape
    nc = _get_nc()
    in_maps = _shard_inputs(x)
    try:
        res = run_bass_kernel_spmd(nc, in_maps, core_ids=list(range(NCORES)))
    except Exception:
        import time

        time.sleep(5)
        res = run_bass_kernel_spmd(nc, in_maps, core_ids=list(range(NCORES)))
    cols = []
    for r in res.results:
        yh = r["y"].reshape(NPANEL * P, N)  # [256, N]
        cols.append(yh.T.astype(np.float32))  # [N, 256]
    return np.concatenate(cols, axis=1)


# revision 3
# speedup vs baseline: 1.0534x; 1.0534x over previous
"""Column-wise RMS normalization on 8 Trainium2 NeuronCores — transposed layout.

Computes y = x * rsqrt(sum(x*x, axis=0) + eps) for x [32768, 2048] f32.

Sharding: column-parallel — each core owns 256 columns (no collectives).
The host casts to fp16 and TRANSPOSES the shard to [2 panels, 128, N]:
panel h of core k holds 128 columns as SBUF partitions, with the full
row axis (N=32768) as the free axis. Every DMA chunk [128, FC] moves
FC*2 bytes CONTIGUOUS per partition (8KB runs), and the column
reduction becomes a free-axis reduction: no TensorE, no PSUM.

Reduction per chunk (1MB, [128,4096]): ACT ACTIVATE(Square,
accum_out) or DVE scalar_tensor_tensor((x*1)*x, accum=sum), both one
pass at ~1 elem/cycle (ACT 3.7us, DVE 4.4us per chunk). DVE also runs
every output mul (2.3us: fp16 tensor_tensor in a [P,g,128] view hits
the 2x-rate mode; a [P,1,128]-replicated scale broadcasts over the
middle axis), so DVE only owns early-arriving square chunks — the
panel-B boundary chain [last-sq -> m6/m7 -> foldB -> Bmul0] stays off
the DMA critical path. Chain: sqrt(u+eps) on ACT + reciprocal on DVE
on [128,1] — per-partition scalars, no broadcast tiles.

DMA: ALL transfers ride the single SP HWDGE queue in strict FIFO
program order [in-A, in-B, out-A, out-B]. A single queue sustains
~420 GB/s, and the FIFO order structurally hides all reduction+chain
latency: out-A's descriptors are consumed only after in-B drains
(~20us after panel A's data lands), out-B after out-A. HBM stays busy
start-to-finish (zero queue gaps on every core); the only exposed
overhead is the fixed runtime preamble/postamble (~16us). Fast cores
run ~96.5us; cores that lose device-level HBM arbitration (8 cores
demanding ~420 GB/s each oversubscribe the device) run ~113-119us —
deliberate run-length throttling to equalize was tested and loses:
the uniform-feasible rate (~320-340 GB/s/core) is no better than what
arbitration victims already get.
"""

import numpy as np

import concourse.bacc as bacc
import concourse.bass as bass
import concourse.tile as tile
from concourse import mybir
from concourse.bass_utils import run_bass_kernel_spmd

N, D = 32768, 2048
EPS = 1e-6
NCORES = 8
P = 128           # partitions = columns per panel
NPANEL = 2        # panels per core (256 cols)
FC = 4096         # free-axis chunk (1MB per DMA / square / mul chunk)
NCH = N // FC     # 8 chunks per panel
H2 = 128          # inner axis of the 3D op views (DVE 2x-rate mode needs
G2 = FC // H2     # a contiguous inner axis; stride-0 only on the middle)

# Square-chunk owners: "A" = ACT one-pass Square+accum, "V" = DVE one-pass
# scalar_tensor_tensor((x*1)*x, accum=sum). Both run at ~1 elem/cycle
# (ACT 3.7us, DVE 4.4us per 4096-chunk); DVE also runs all output muls
# (2.3us each, 2x mode), so it only takes early-arriving square chunks —
# the panel-B boundary chain is [last-sq -> m6/m7 -> foldB -> Bmul0].
OWN_A = ["V", "A", "A", "V", "A", "A", "V", "A"]
OWN_B = ["V", "A", "A", "V", "A", "A", "A", "A"]
# Output-mul owners: "V" = DVE broadcast mul, "A" = ACT Copy-with-scale.
MUL_A = ["V"] * NCH
MUL_B = ["V"] * NCH

_NC = None


def _build() -> bass.Bass:
    nc = bacc.Bacc("TRN2", target_bir_lowering=False, enable_partition_id=False)
    x = nc.dram_tensor("x", [NPANEL, P, N], mybir.dt.float16, kind="ExternalInput")
    y = nc.dram_tensor("y", [NPANEL, P, N], mybir.dt.float16, kind="ExternalOutput")

    with tile.TileContext(nc) as tc:
        with (
            tc.tile_pool(name="cache", bufs=1) as cachep,
            tc.tile_pool(name="consts", bufs=1) as consts,
            tc.tile_pool(name="sq", bufs=2) as sqp,
            tc.tile_pool(name="outs", bufs=4) as outp,
            tc.tile_pool(name="scale", bufs=1) as scalep,
        ):
            xc = cachep.tile([P, NPANEL, N], mybir.dt.float16)
            eps_t = consts.tile([P, 1], mybir.dt.float32)
            nc.vector.memset(eps_t, EPS)
            # Pre-warm ACT function tables off the critical path.
            warm = consts.tile([1, 1], mybir.dt.float32)
            nc.scalar.activation(
                out=warm, in_=eps_t[0:1, 0:1],
                func=mybir.ActivationFunctionType.Square,
            )
            nc.scalar.activation(
                out=warm, in_=eps_t[0:1, 0:1],
                func=mybir.ActivationFunctionType.Sqrt,
            )

            parts = scalep.tile([P, NPANEL, NCH], mybir.dt.float32)
            s32 = [scalep.tile([P, 1], mybir.dt.float32, name=f"s32_{h}")
                   for h in range(NPANEL)]
            # Scale replicated along a 128-wide inner axis so output muls can
            # broadcast over the MIDDLE axis with a contiguous inner axis
            # (DVE 2x fp16 mode), like the baseline's [P,1,H] pattern.
            smax = [scalep.tile([P, 1, H2], mybir.dt.float16, name=f"smax{h}")
                    for h in range(NPANEL)]

            def in_panel(h):
                for c in range(NCH):
                    cs = slice(c * FC, (c + 1) * FC)
                    nc.sync.dma_start(out=xc[:, h, cs], in_=x[h, :, cs])

            def sq_chunk(h, c, owner):
                cs = slice(c * FC, (c + 1) * FC)
                part = parts[:, h, c : c + 1]
                if owner == "A":
                    scr = sqp.tile([P, FC], mybir.dt.float16, tag="scrA", bufs=1)
                    nc.scalar.activation(
                        out=scr, in_=xc[:, h, cs],
                        func=mybir.ActivationFunctionType.Square,
                        accum_out=part,
                    )
                else:
                    xv = xc[:, h, cs].rearrange("p (g i) -> p g i", i=H2)
                    scr = sqp.tile([P, G2, H2], mybir.dt.float16, tag="scrV", bufs=1)
                    nc.vector.scalar_tensor_tensor(
                        out=scr, in0=xv, scalar=1.0, in1=xv,
                        op0=mybir.AluOpType.mult, op1=mybir.AluOpType.mult,
                        accum_out=part,
                    )

            def chain(h):
                u = scalep.tile([P, 1], mybir.dt.float32, name=f"u{h}")
                nc.vector.reduce_sum(
                    u, parts[:, h : h + 1, :], axis=mybir.AxisListType.X
                )
                t = scalep.tile([P, 1], mybir.dt.float32, name=f"t{h}")
                nc.scalar.activation(
                    out=t, in_=u,
                    func=mybir.ActivationFunctionType.Sqrt,
                    bias=eps_t[:, 0:1], scale=1.0,
                )
                nc.vector.reciprocal_approx_fast(out=s32[h], in_=t)
                nc.vector.tensor_copy(
                    smax[h], s32[h][:, 0:1].to_broadcast((P, 1, H2))
                )

            def out_chunk(h, c, owner, split=1):
                cs = slice(c * FC, (c + 1) * FC)
                ot = outp.tile([P, G2, H2], mybir.dt.float16, tag="ot", bufs=6)
                if owner == "A":
                    nc.scalar.activation(
                        out=ot, in_=xc[:, h, cs].rearrange("p (g i) -> p g i", i=H2),
                        func=mybir.ActivationFunctionType.Copy,
                        scale=s32[h][:, 0:1],
                    )
                else:
                    xv = xc[:, h, cs].rearrange("p (g i) -> p g i", i=H2)
                    nc.vector.tensor_mul(
                        ot, xv, smax[h][:, :, :].to_broadcast((P, G2, H2))
                    )
                otf = ot[:, :, :].rearrange("p g i -> p (g i)")
                # split>1 pipelines the final chunk's completion drain.
                fs = FC // split
                for k in range(split):
                    ks = slice(c * FC + k * fs, c * FC + (k + 1) * fs)
                    nc.sync.dma_start(out=y[h, :, ks], in_=otf[:, k * fs : (k + 1) * fs])

            # Panel A input + squares.
            in_panel(0)
            for c in range(NCH):
                sq_chunk(0, c, OWN_A[c])
            chain(0)
            # Panel B input (SP FIFO: behind all of in-A).
            in_panel(1)
            # Interleave panel-B squares with panel-A outputs.
            for c in range(NCH):
                sq_chunk(1, c, OWN_B[c])
                out_chunk(0, c, MUL_A[c])
            chain(1)
            # Panel B outputs (SP FIFO: behind out-A). The final chunk's DMA
            # is split so the end-of-kernel completion drain overlaps earlier
            # sub-transfers.
            for c in range(NCH):
                out_chunk(1, c, MUL_B[c], split=(4 if c == NCH - 1 else 1))
    nc.compile()
    return nc


def _get_nc() -> bass.Bass:
    global _NC
    if _NC is None:
        _NC = _build()
    return _NC


def _shard_inputs(x: np.ndarray) -> list[dict]:
    xt = np.ascontiguousarray(x.astype(np.float16).T)  # [D, N]
    out = []
    for i in range(NCORES):
        sh = xt[i * NPANEL * P : (i + 1) * NPANEL * P]  # [256, N]
        out.append({"x": sh.reshape(NPANEL, P, N)})
    return out


def kernel(x) -> np.ndarray:
    x = np.asarray(x, dtype=np.float32)
    assert x.shape == (N, D), x.shape
    nc = _get_nc()
    in_maps = _shard_inputs(x)
    try:
        res = run_bass_kernel_spmd(nc, in_maps, core_ids=list(range(NCORES)))
    except Exception:
        import time

        time.sleep(5)
        res = run_bass_kernel_spmd(nc, in_maps, core_ids=list(range(NCORES)))
    cols = []
    for r in res.results:
        yh = r["y"].reshape(NPANEL * P, N)  # [256, N]
        cols.append(yh.T.astype(np.float32))  # [N, 256]
    return np.concatenate(cols, axis=1)
